# revision 13
# baseline (speedup 1.0000x reference)
"""Trainium2 Bass kernel for the Comatch retrieval problem.

Math: the reference's log_softmax + gumbel_softmax(hard=True) + straight-through
pipeline is numerically equivalent (to fp rounding) to a hard argmax selection:

    z[q,d,m,n]  = sim[q,d,m,n]/TEMP + BIAS[d,n] + gumbel[q,d,m,n]
                  (BIAS = (1-d_mask)*NEG/TEMP; LSE of log_softmax is constant
                   along n so it never changes the argmax; softmax is monotone)
    idx         = argmax_n z
    scores[q,d] = sum_m q_mask[q,m] * sim[q,d,m,idx]

On device (per core, d-axis sharded 200 -> 8*25):
  - PE projects Q@W and D@W with h-major (host-pretransposed) operands,
    row norms via ones-matmul column sums, 1/sqrt via ACT Ln/Exp.
  - per (qm-chunk of 128, d): PE matmul gives t = sim/TEMP in PSUM;
    DVE tensor_tensor_reduce computes z = t + g' (g' = gumbel + BIAS,
    prefolded on host) and its row max v in ONE pass;
    DVE scalar_tensor_tensor computes (z >= v) * g' and its row sum s in ONE
    pass.  Selected sim/TEMP = v - s.
  - scores = (q_mask*TEMP)^T @ (v - s) via a tiny PE matmul per chunk.
"""

import math
import os
import sys

import numpy as np

for _p in ("/opt/trn_rl_repo", os.path.expanduser("~/.axon_site/_ro/trn_rl_repo")):
    if os.path.isdir(_p) and _p not in sys.path:
        sys.path.append(_p)

from contextlib import ExitStack

import concourse.bass as bass
import concourse.mybir as mybir
import concourse.tile as tile
from concourse.bass import ds, ts
from concourse import bacc
from concourse.bass_utils import run_bass_kernel_spmd

F32 = mybir.dt.float32
AF = mybir.ActivationFunctionType
OP = mybir.AluOpType

NQ, M, ND, N, H, DIM = 32, 32, 200, 160, 768, 128
TEMP = 0.1
NEG = -10000.0

NCORES = 8
DLOC = ND // NCORES          # 25 docs per core
QM = NQ * M                  # 1024 query tokens
P = 128                      # partitions
NCH = QM // P                # 8 qm chunks
KH = H // P                  # 6 contraction chunks
FD = DLOC * N                # 4000 doc tokens per core
G_SIZES = [3] * 8 + [1]      # d-groups per 480-col psum tile

_CACHE = {}


def _trace_kernel(nc, repeat=1):
    qt_d = nc.dram_tensor("qt", [H, QM], F32, kind="ExternalInput").ap()
    dt_d = nc.dram_tensor("dt", [H, FD], F32, kind="ExternalInput").ap()
    w_d = nc.dram_tensor("w", [H, DIM], F32, kind="ExternalInput").ap()
    gb_d = nc.dram_tensor("gb", [NCH, P, DLOC, N], F32, kind="ExternalInput").ap()
    qw_d = nc.dram_tensor("qw", [NCH, P, 4], F32, kind="ExternalInput").ap()
    id_d = nc.dram_tensor("ident", [P, P], F32, kind="ExternalInput").ap()
    out_d = nc.dram_tensor("scores", [NCH, 4, DLOC], F32, kind="ExternalOutput").ap()

    # per-chunk group structure: two PSUM waves; per group, which engine adds g
    WAVES = [list(range(0, 5)), list(range(5, 9))]
    GADD = {0: "pe", 1: "pe", 2: "gps", 3: "gps", 4: "gps",
            5: "pe", 6: "gps", 7: "gps", 8: "dve"}

    with tile.TileContext(nc) as tc, ExitStack() as ctx:
        const = ctx.enter_context(tc.tile_pool(name="const", bufs=1))
        gpool = ctx.enter_context(tc.tile_pool(name="gpool", bufs=2))
        zpool = ctx.enter_context(tc.tile_pool(name="zpool", bufs=4))
        vpool = ctx.enter_context(tc.tile_pool(name="vpool", bufs=2))
        opool = ctx.enter_context(tc.tile_pool(name="opool", bufs=2))

        ones_col = const.tile([P, 1], F32)
        nc.vector.memset(ones_col, 1.0)
        ones_row = const.tile([1, P], F32)
        nc.vector.memset(ones_row, 1.0)
        ln10 = const.tile([1, 1], F32)
        nc.vector.memset(ln10, float(math.log(1.0 / TEMP)))

        for _rep in range(repeat):
            w_sb = const.tile([P, KH, DIM], F32, tag="w")
            nc.sync.dma_start(out=w_sb, in_=w_d.rearrange("(k p) d -> p k d", p=P))
            qw_sb = const.tile([P, NCH, 4], F32, tag="qw")
            nc.sync.dma_start(out=qw_sb, in_=qw_d.rearrange("c p j -> p c j"))
            id_sb = const.tile([P, P], F32, tag="ident")
            nc.sync.dma_start(out=id_sb, in_=id_d)
            qp_sb = const.tile([P, QM], F32, tag="qp")    # (Qp/TEMP)^T [dim, qm]
            dpt_sb = const.tile([P, FD], F32, tag="dpt")  # Dp^T [dim, d*n]

            # ---- Q projection + l2norm (scaled by 1/TEMP) ---------------
            # ACT does only batched Ln -> Exp (one table set each, no thrash)
            with tc.tile_pool(name="projq", bufs=2) as projq, \
                 tc.tile_pool(name="ppq", bufs=2, space="PSUM") as ppq, \
                 tc.tile_pool(name="ppq1", bufs=2, space="PSUM") as ppq1:
                qt_sb = projq.tile([P, KH, QM], F32, tag="qt")
                nc.sync.dma_start(out=qt_sb,
                                  in_=qt_d.rearrange("(k p) m -> p k m", p=P))
                pq_all = projq.tile([P, QM], F32, tag="pqall", bufs=1)
                lnq = projq.tile([1, QM], F32, tag="lnq", bufs=1)
                r10 = projq.tile([1, QM], F32, tag="r10", bufs=1)
                for h in range(2):
                    pq = ppq.tile([P, 512], F32, tag="pq")
                    for k in range(KH):
                        nc.tensor.matmul(pq, lhsT=w_sb[:, k, :],
                                         rhs=qt_sb[:, k, ts(h, 512)],
                                         start=(k == 0), stop=(k == KH - 1))
                    nc.vector.tensor_copy(pq_all[:, ts(h, 512)], pq)
                    sq = projq.tile([P, 512], F32, tag="sq")
                    nc.gpsimd.tensor_mul(sq, pq_all[:, ts(h, 512)],
                                         pq_all[:, ts(h, 512)])
                    n2p = ppq1.tile([1, 512], F32, tag="n2q")
                    nc.tensor.matmul(n2p, lhsT=ones_col, rhs=sq, start=True, stop=True)
                    nc.scalar.activation(lnq[:, ts(h, 512)], n2p, AF.Ln)
                nc.scalar.activation(r10, lnq, AF.Exp, scale=-0.5, bias=ln10[:, :])
                for h in range(2):
                    rbq = ppq1.tile([P, 512], F32, tag="rbq")
                    nc.tensor.matmul(rbq, lhsT=ones_row, rhs=r10[:, ts(h, 512)],
                                     start=True, stop=True)
                    nc.vector.tensor_mul(qp_sb[:, ts(h, 512)], rbq,
                                         pq_all[:, ts(h, 512)])

            # ---- D projection + l2norm ----------------------------------
            with tc.tile_pool(name="projd", bufs=2) as projd, \
                 tc.tile_pool(name="ppd", bufs=2, space="PSUM") as ppd, \
                 tc.tile_pool(name="ppd1", bufs=2, space="PSUM") as ppd1:
                dt_r = dt_d.rearrange("(k p) n -> p k n", p=P)
                pd_all = projd.tile([P, FD], F32, tag="pdall", bufs=1)
                lnd = projd.tile([1, FD], F32, tag="lnd", bufs=1)
                rd = projd.tile([1, FD], F32, tag="rd", bufs=1)
                for q8 in range(8):
                    dtq = projd.tile([P, KH, 500], F32, tag="dtq")
                    nc.sync.dma_start(out=dtq, in_=dt_r[:, :, ds(q8 * 500, 500)])
                    for j in range(1):
                        off = q8 * 500
                        pd = ppd.tile([P, 500], F32, tag="pd")
                        for k in range(KH):
                            nc.tensor.matmul(pd, lhsT=w_sb[:, k, :],
                                             rhs=dtq[:, k, :],
                                             start=(k == 0), stop=(k == KH - 1))
                        nc.vector.tensor_copy(pd_all[:, ds(off, 500)], pd)
                        sqd = projd.tile([P, 500], F32, tag="sqd")
                        nc.gpsimd.tensor_mul(sqd, pd_all[:, ds(off, 500)],
                                             pd_all[:, ds(off, 500)])
                        n2d = ppd1.tile([1, 500], F32, tag="n2d")
                        nc.tensor.matmul(n2d, lhsT=ones_col, rhs=sqd,
                                         start=True, stop=True)
                        nc.scalar.activation(lnd[:, ds(off, 500)], n2d, AF.Ln)
                nc.scalar.activation(rd, lnd, AF.Exp, scale=-0.5)
                for t8 in range(8):
                    off = t8 * 500
                    rbd = ppd1.tile([P, 500], F32, tag="rbd")
                    nc.tensor.matmul(rbd, lhsT=ones_row, rhs=rd[:, ds(off, 500)],
                                     start=True, stop=True)
                    nc.vector.tensor_mul(dpt_sb[:, ds(off, 500)], rbd,
                                         pd_all[:, ds(off, 500)])

            # ---- main phase ---------------------------------------------
            with tc.tile_pool(name="pp_t", bufs=5, space="PSUM") as pp_t, \
                 tc.tile_pool(name="pp_sc", bufs=2, space="PSUM") as pp_sc:
                for c in range(NCH):
                    gbt = gpool.tile([P, DLOC, N], F32, tag="gb")
                    nc.sync.dma_start(out=gbt, in_=gb_d[c])
                    zsb = gpool.tile([P, DLOC, N], F32, tag="zsb")
                    for wave in WAVES:
                        tps = {}
                        for g_i in wave:
                            gsz = G_SIZES[g_i]
                            d0 = sum(G_SIZES[:g_i])
                            tp = pp_t.tile([P, G_SIZES[0] * N], F32, tag="t")
                            nc.tensor.matmul(tp[:, : gsz * N],
                                             lhsT=qp_sb[:, ts(c, P)],
                                             rhs=dpt_sb[:, ds(d0 * N, gsz * N)],
                                             start=True, stop=GADD[g_i] != "pe")
                            tps[g_i] = tp
                        for g_i in wave:
                            if GADD[g_i] != "pe":
                                continue
                            gsz = G_SIZES[g_i]
                            d0 = sum(G_SIZES[:g_i])
                            nc.tensor.matmul(tps[g_i][:, : gsz * N], lhsT=id_sb,
                                             rhs=gbt[:, ds(d0, gsz), :],
                                             start=False, stop=True)
                        for g_i in wave:
                            gsz = G_SIZES[g_i]
                            d0 = sum(G_SIZES[:g_i])
                            zslice = zsb[:, ds(d0, gsz), :]
                            nc.scalar.copy(zslice, tps[g_i][:, : gsz * N])
                            if GADD[g_i] == "gps":
                                nc.gpsimd.tensor_add(zslice, zslice,
                                                     gbt[:, ds(d0, gsz), :])
                            elif GADD[g_i] == "dve":
                                nc.vector.tensor_add(zslice, zslice,
                                                     gbt[:, ds(d0, gsz), :])
                    v = vpool.tile([P, DLOC], F32, tag="v")
                    nc.vector.tensor_reduce(out=v, in_=zsb,
                                            axis=mybir.AxisListType.X, op=OP.max)
                    s = vpool.tile([P, DLOC], F32, tag="s")
                    for d in range(DLOC):
                        wt = zpool.tile([P, N], F32, tag="w")
                        nc.vector.scalar_tensor_tensor(
                            out=wt, in0=zsb[:, d, :], scalar=v[:, ds(d, 1)],
                            in1=gbt[:, d, :],
                            op0=OP.is_ge, op1=OP.mult, accum_out=s[:, ds(d, 1)])
                    sel = vpool.tile([P, DLOC], F32, tag="sel")
                    nc.vector.tensor_sub(sel, v, s)
                    scp = pp_sc.tile([4, DLOC], F32, tag="sc")
                    nc.tensor.matmul(scp, lhsT=qw_sb[:, c, :], rhs=sel, start=True,
                                     stop=True)
                    sco = opool.tile([4, DLOC], F32, tag="sco")
                    nc.scalar.copy(sco, scp)
                    nc.sync.dma_start(out=out_d[c], in_=sco)
    return nc


def _get_nc(repeat=1):
    key = ("nc", repeat)
    if key not in _CACHE:
        nc = bacc.Bacc("TRN2", target_bir_lowering=False, debug=False)
        _trace_kernel(nc, repeat=repeat)
        nc.compile()
        _CACHE[key] = nc
    return _CACHE[key]


def _prep_inputs(Q, D, q_mask, d_mask, gumbel_noise, W_proj):
    Q = np.ascontiguousarray(Q, dtype=np.float32)
    D = np.ascontiguousarray(D, dtype=np.float32)
    q_mask = np.asarray(q_mask, dtype=np.float32)
    d_mask = np.asarray(d_mask, dtype=np.float32)
    W = np.ascontiguousarray(W_proj, dtype=np.float32)

    Qt = np.ascontiguousarray(Q.reshape(QM, H).T)                 # [768, 1024]
    qwf = q_mask * np.float32(TEMP)                               # fold 1/TEMP back out
    qw = np.zeros((NCH, P, 4), np.float32)
    for c in range(NCH):
        for j in range(4):
            qw[c, j * 32:(j + 1) * 32, j] = qwf[4 * c + j]

    ident = np.eye(P, dtype=np.float32)
    in_maps = []
    for k in range(NCORES):
        dsl = slice(k * DLOC, (k + 1) * DLOC)
        Dt = np.ascontiguousarray(D[dsl].reshape(DLOC * N, H).T)  # [768, 4000]
        bias = (1.0 - d_mask[dsl]) * np.float32(NEG / TEMP)       # [25, 160]
        gb = gumbel_noise[:, dsl].astype(np.float32).transpose(0, 2, 1, 3)
        gb = gb + bias[None, None, :, :]                          # [q, m, d, n]
        gb = np.ascontiguousarray(gb.reshape(NCH, P, DLOC, N))
        in_maps.append({"qt": Qt, "dt": Dt, "w": W, "gb": gb, "qw": qw,
                        "ident": ident})
    return in_maps


def _postprocess(results):
    full = np.empty((NQ, ND), np.float32)
    for k, res in enumerate(results):
        full[:, k * DLOC:(k + 1) * DLOC] = res["scores"].reshape(NQ, DLOC)
    return full


def kernel(Q, D, q_mask, d_mask, gumbel_noise, W_proj):
    nc = _get_nc()
    in_maps = _prep_inputs(Q, D, q_mask, d_mask, gumbel_noise, W_proj)
    res = run_bass_kernel_spmd(nc, in_maps, core_ids=list(range(NCORES)))
    return _postprocess(res.results)


def run_repeat(inputs, repeat=1):
    """Run the program with the body repeated `repeat` times; returns scores."""
    nc = _get_nc(repeat)
    in_maps = _prep_inputs(**inputs)
    res = run_bass_kernel_spmd(nc, in_maps, core_ids=list(range(NCORES)))
    return _postprocess(res.results)
